# revision 60
# baseline (speedup 1.0000x reference)
"""Trainium2 Bass kernel for nn_CSFlow (RAFT-style correlation pyramid lookup).

v10: separable blend weights + grouped blends + lean DMA schedule.

Structure (per core, one 24h x 80w quadrant of one batch):
  - corr(q, pos) = <fmap1[:, q], fmap2[:, pos]> / sqrt(D); pooling folded into
    pooled fmap2 levels (linearity), one matmul per pyramid level slice.
  - 15 tiles of 8h x 16w query blocks (128 queries = partitions). Per tile a
    static per-level x-range bbox (2792 cols total) is matmul'd, copied
    PSUM->SBUF f16, written to a DRAM scratch, and 4 indirect gathers pull
    each query's band (one per level; HW honors one offset per partition).
  - Blends use separable weights: per (level, tile, query) vectors
    a0/a1[9] (inner-tap weight x validity) and b0/b1[9] (outer-tap), applied
    as broadcast tensor_tensor ops batched over groups of tiles:
      h[t,r,j]  = g0*a0 + g1*a1   (3 ops per level-group)
      out[t,a,j] = h[.,a,.]*b0 + h[.,a+1,.]*b1  (3 ops, into persistent out)
  - Outputs accumulate in one SBUF tile; one DMA per tile group.
  - Guard zones of the scratch are zeroed in one DMA upfront.
  - Input loads are ordered so tile 0's write (the head of the serial
    gather chain on the Pool engine) lands as early as possible.

Output channel order per level block l: a*9+j where a = x-tap, j = y-tap for
L0-2; L3 blocks are (y-tap major) and host transposes them.
"""

import numpy as np

import concourse.bass as bass
import concourse.mybir as mybir
import concourse.tile as tile
from concourse import bacc
from concourse.bass_utils import run_bass_kernel_spmd

# problem shape (hardcoded per harness contract)
B, D, H, W = 2, 256, 48, 160
NCORES = 8
P = 128
NT = 15                      # tiles per core (3x5 blocks of 8h x 16w)
NLVL = 4
QPC = NT * P                 # 1920 queries per core (24h x 80w quadrant)
LH = [48, 24, 12, 6]
LW = [160, 80, 40, 20]

# L0+L1 are served by ONE merged gather per (tile, query): a 20x20
# L0-granularity band anchored at (2*floor(cx/2)-8, 2*floor(cy/2)-8) covers
# both L0's 10x10 window and the 2x2-pool preimage of L1's 10x10 window.
# L1 values are produced on-device by pooling the band; the anchor parity
# (px, py) is absorbed into 3-tap blend stages for L0.
SY = [48, 24, 12, 20]        # L0m inner stride: full 48 rows (L1 preimage)
SXB0 = 54                    # L0m bbox x-extent
SXB = [54, 28, 20, 6]        # bbox outer extent (L2/L3 unchanged)
COLS = [2592, 0, 240, 120]   # bbox positions per level (L1 merged into L0)
# per-partition section order [L2 240 | L3 120 | padh 144 | L0m 2592 |
# padt 40]: zero pads absorb the merged band's worst-case under/overrun so
# every read lands on written scratch -> no guard zones. Pads are zeroed in
# the staging tiles once per ring buffer and rewritten with each tile.
PADH = 144
PADT = 40
SCOLS = 240 + 120 + PADH + 2592 + PADT  # 3136 per-partition cols
SOFFR = [360 + PADH, -1, 0, 240]  # per-partition section offsets by level

WXC = [118, 0, 40, 120]      # windowed-f2 x-columns per level (L1 merged away)
# f2 layout: [L0 region 118x48 | 5 merged L2|L3 blocks of 360]
LOFF23 = 118 * 48            # 5664
NPOSW = LOFF23 + 5 * 360     # 7464
WPAD = [14, 12, 10, 0]       # f2-window left margin vs quadrant x-base
ML = [14, 10, 8, 0]          # bbox left margin vs block x-origin (per level)

SCR1 = P * SCOLS  # no guard zones (see SOFFR comment)
BLEN = [19 * 48 + 20, 0, 118, 190]  # gather band lengths (L0m: 20x20 block)
BOFF = [0, -1, 932, 1052]    # band section offsets within a tile's band
BTOT = 1252  # incl. view-overhang padding (L2 +2 and L3's [10,20] window)
GLVL = [0, 2, 3]             # gathered levels

TG = [5, 5, 3, 2]            # blend/output tile groups (last small = short tail)
GOF = [0, 5, 10, 13]

F16 = mybir.dt.float16
F32 = mybir.dt.float32
BF16 = mybir.dt.bfloat16
I32 = mybir.dt.int32

MM_CHUNK = 512
# psum chunks: (name, [(level, col-off-in-level, psum-off, size)...], total,
#               copy engine: 0=DVE 1=ACT)
# psum chunks; stag cols via SOFFR: c0-c2 [504:3096] (L0m), c3 [0:360]
# (L2|L3). All copies on ACT; write A ([504:3136], after c2, incl. padt)
# unblocks the L0m gather; write B ([0:504], after c3, incl. padh) unblocks
# L2/L3.
CHUNKS = [
    ("c0", [(0, 0, 0, 1024)], 1024),
    ("c1", [(0, 1024, 0, 1024)], 1024),
    ("c2", [(0, 2048, 0, 544)], 544),
    ("c3", [(2, 0, 0, 360)], 360),
]


def _f2_slice_start(l, bw):
    """Static f2-window column offset for tile column bw (0..4), level l."""
    if l == 0:
        return (16 * bw) * SY[0]
    # merged L2-window + L3-map block, replicated per tile column
    return LOFF23 + 360 * bw


def build_nc(repeat=1):
    nc = bacc.Bacc("TRN2", target_bir_lowering=False, debug=False)

    # f1 is (t, k)-blocked so every load and matmul lhsT slice is contiguous
    f1t = nc.dram_tensor("f1t", [P, NT * 2 * P], BF16, kind="ExternalInput")
    f2t = nc.dram_tensor("f2t", [P, 2 * NPOSW], BF16, kind="ExternalInput")
    # l-major t-contig (l in GLVL order: L0m, L2, L3)
    idxt = nc.dram_tensor("idxt", [P, 3 * NT], I32, kind="ExternalInput")
    # separable blend weights, l-major t-contig:
    #   L0m: alpha[3][9] beta[3][9] (54); L1/L2/L3: a0 a1 b0 b1 (36 each)
    abt = nc.dram_tensor("abt", [P, NT * 162], F16, kind="ExternalInput")
    # l-major: [P, NLVL, NT, 81]
    outp = nc.dram_tensor("outp", [P, NLVL * NT * 81], F16, kind="ExternalOutput")

    with tile.TileContext(nc) as tc:
        with (
            tc.tile_pool(name="dram", bufs=1, space="DRAM") as dpool,
            tc.tile_pool(name="const", bufs=1) as cpool,
            tc.tile_pool(name="stag", bufs=3) as stpool,
            tc.tile_pool(name="bands", bufs=2) as bpool,
            tc.tile_pool(name="blend", bufs=4) as blpool,
            tc.tile_pool(name="psum", bufs=4, space="PSUM") as pspool,
        ):
            # one DRAM scratch tensor PER TILE: an indirect gather's source AP
            # must be a whole tensor (offset 0), and a shared tensor would give
            # every later write a false WAR dependency on all prior gathers.
            scrt = [dpool.tile([SCR1], F16, name=f"scrt{t}") for t in range(NT)]

            # f1sb mirrors f1t's (t, k)-blocked layout
            f1sb = cpool.tile([P, NT * 2 * P], BF16)
            f2sb = cpool.tile([P, 2 * NPOSW], BF16)
            idx_sb = cpool.tile([P, 3 * NT], I32)
            ab_sb = cpool.tile([P, NT * 162], F16)
            # l-major: [P, NLVL, NT, 81]
            out_sb = cpool.tile([P, NLVL * NT * 81], F16)

            # --- critical-path loads first (finest useful granularity so
            # tile 0's chunks unblock ASAP); every load is a contiguous
            # [128, N] row copy. ---
            def f2load(eng, a, b):
                for k in range(2):
                    eng.dma_start(
                        f2sb[:, k * NPOSW + a : k * NPOSW + b],
                        f2t[:, k * NPOSW + a : k * NPOSW + b],
                    )

            # tile 0's critical path: f2 L0a on the otherwise-idle sync queue
            # so its completion isn't slowed by concurrent bulk loads
            f2load(nc.sync, 0, 1024)
            nc.sync.dma_start(f1sb[:, 0 : 4 * P], f1t[:, 0 : 4 * P])  # tiles 0-1
            nc.sync.dma_start(idx_sb[:], idxt[:])
            # tiles 0-1 remaining L0m cols
            f2load(nc.scalar, 1024, 3360)
            # merged-L23 blocks (tile 0-4 chunk c3)
            f2load(nc.scalar, LOFF23, NPOSW)

            # rest of f1 + f2 + weights, deprioritized behind the pipeline
            def deferred_loads(t):
                if t == 1:
                    with tc.high_priority(offset=-100):
                        nc.sync.dma_start(
                            f1sb[:, 4 * P : NT * 2 * P], f1t[:, 4 * P : NT * 2 * P]
                        )
                        f2load(nc.scalar, 3360, LOFF23)
                elif t == 2:
                    with tc.high_priority(offset=-100):
                        nc.scalar.dma_start(ab_sb[:], abt[:])

            import contextlib

            rep_ctx = tc.For_i(0, repeat, 1) if repeat > 1 else contextlib.nullcontext()

            def stage_mm(t):
                """matmuls -> psum -> f16 staging (ACT) -> two scratch writes."""
                # deferred loads must be EMITTED before this tile's readers
                # or the read-before-write flips the hazard direction
                deferred_loads(t)
                bw = t % 5
                stag = stpool.tile([P, SCOLS], F16, name="stag")
                if t < 3:
                    # zero the pad columns once per ring buffer; copies never
                    # touch them, so they persist for later tiles
                    nc.vector.memset(stag[:, 360 : 360 + PADH], 0.0)
                    nc.vector.memset(stag[:, SCOLS - PADT : SCOLS], 0.0)
                wv = scrt[t][:].rearrange("(p x) -> p x", x=SCOLS)
                for ci, (nm, parts, csz) in enumerate(CHUNKS):
                    ps = pspool.tile([P, 1024], F32, name="cps")[:, :csz]
                    for k in range(2):
                        for (l, coff, poff, sz) in parts:
                            fs = k * NPOSW + _f2_slice_start(l, bw) + coff
                            for soff in range(0, sz, MM_CHUNK):
                                ssz = min(MM_CHUNK, sz - soff)
                                nc.tensor.matmul(
                                    ps[:, poff + soff : poff + soff + ssz],
                                    f1sb[:, (2 * t + k) * P : (2 * t + k + 1) * P],
                                    f2sb[:, fs + soff : fs + soff + ssz],
                                    start=(k == 0),
                                    stop=(k == 1),
                                )
                    soff0 = SOFFR[parts[0][0]] + parts[0][1]
                    nc.scalar.copy(stag[:, soff0 : soff0 + csz], ps)
                    if ci == 2:
                        # L0m staged -> write A (incl. padt), unblocking its
                        # gather
                        nc.sync.dma_start(wv[:, 504:SCOLS], stag[:, 504:SCOLS])
                    elif ci == 3:
                        # L2|L3 (+padh) -> write B
                        nc.sync.dma_start(wv[:, 0:504], stag[:, 0:504])

            def stage_gather_tile(t, band, tt):
                """indirect gathers (L0m, L2, L3) for tile t into band slot tt."""
                for gi, l in enumerate(GLVL):
                    nc.gpsimd.indirect_dma_start(
                        out=band[
                            :, tt * BTOT + BOFF[l] : tt * BTOT + BOFF[l] + BLEN[l]
                        ],
                        out_offset=None,
                        in_=scrt[t][:].unsqueeze(1),
                        in_offset=bass.IndirectOffsetOnAxis(
                            ap=idx_sb[:, gi * NT + t : gi * NT + t + 1],
                            axis=0,
                        ),
                        element_offset=0,
                    )

            # ab sections: L0m [0:54*NT], L1 [54*NT:90*NT], L2 [...], L3
            ab0v = ab_sb[:, 0 : 54 * NT].rearrange("p (c v) -> p c v", v=54)
            abv = ab_sb[:, 54 * NT :].rearrange("p (c v) -> p c v", v=36)

            def blend2tap(E, g0, g1, T, c0i, os):
                """standard separable 2-tap blend into out_sb at offset os."""
                a0 = abv[:, c0i : c0i + T, 0:9].unsqueeze(2).to_broadcast(
                    (P, T, 10, 9)
                )
                a1 = abv[:, c0i : c0i + T, 9:18].unsqueeze(2).to_broadcast(
                    (P, T, 10, 9)
                )
                b0 = abv[:, c0i : c0i + T, 18:27].unsqueeze(3).to_broadcast(
                    (P, T, 9, 9)
                )
                b1 = abv[:, c0i : c0i + T, 27:36].unsqueeze(3).to_broadcast(
                    (P, T, 9, 9)
                )
                h = blpool.tile([P, 5 * 90], F16, name="h")[:, : T * 90]
                h2 = blpool.tile([P, 5 * 90], F16, name="h2")[:, : T * 90]
                hv = h.rearrange("p (t r j) -> p t r j", r=10, j=9)
                h2v = h2.rearrange("p (t r j) -> p t r j", r=10, j=9)
                E.tensor_tensor(out=hv, in0=g0, in1=a0, op=mybir.AluOpType.mult)
                E.tensor_tensor(out=h2v, in0=g1, in1=a1, op=mybir.AluOpType.mult)
                E.tensor_add(out=h, in0=h, in1=h2)
                ov = out_sb[:, os : os + T * 81].rearrange(
                    "p (t a j) -> p t a j", a=9, j=9
                )
                o2 = blpool.tile([P, 5 * 81], F16, name="o2")[:, : T * 81]
                o2v = o2.rearrange("p (t a j) -> p t a j", a=9, j=9)
                E.tensor_tensor(
                    out=ov, in0=hv[:, :, 0:9, :], in1=b0, op=mybir.AluOpType.mult
                )
                E.tensor_tensor(
                    out=o2v, in0=hv[:, :, 1:10, :], in1=b1, op=mybir.AluOpType.mult
                )
                E.tensor_add(out=ov, in0=ov, in1=o2v)

            def stage_blend(g, band):
                T = TG[g]
                t0 = GOF[g]
                E = nc.vector
                bg = band[:].rearrange("p (t x) -> p t x", t=T)
                # --- merged L0: 3-tap separable blend over the 20x20 band ---
                # view envelope 20*48=960 spills into the L2 band section; only
                # r<20, s<16 are read
                B = bg[:, :, 0:960].rearrange("p t (r s) -> p t r s", s=48)
                al = [
                    ab0v[:, t0 : t0 + T, 9 * k : 9 * k + 9]
                    .unsqueeze(2)
                    .to_broadcast((P, T, 11, 9))
                    for k in range(3)
                ]
                be = [
                    ab0v[:, t0 : t0 + T, 27 + 9 * k : 36 + 9 * k]
                    .unsqueeze(3)
                    .to_broadcast((P, T, 9, 9))
                    for k in range(3)
                ]
                h = blpool.tile([P, 5 * 99], F16, name="hm")[:, : T * 99]
                h2 = blpool.tile([P, 5 * 99], F16, name="hm2")[:, : T * 99]
                hv = h.rearrange("p (t r j) -> p t r j", r=11, j=9)
                h2v = h2.rearrange("p (t r j) -> p t r j", r=11, j=9)
                E.tensor_tensor(
                    out=hv, in0=B[:, :, 4:15, 4:13], in1=al[0],
                    op=mybir.AluOpType.mult,
                )
                E.tensor_tensor(
                    out=h2v, in0=B[:, :, 4:15, 5:14], in1=al[1],
                    op=mybir.AluOpType.mult,
                )
                E.tensor_add(out=h, in0=h, in1=h2)
                E.tensor_tensor(
                    out=h2v, in0=B[:, :, 4:15, 6:15], in1=al[2],
                    op=mybir.AluOpType.mult,
                )
                E.tensor_add(out=h, in0=h, in1=h2)
                os0 = (0 * NT + t0) * 81
                ov = out_sb[:, os0 : os0 + T * 81].rearrange(
                    "p (t a j) -> p t a j", a=9, j=9
                )
                o2 = blpool.tile([P, 5 * 81], F16, name="om2")[:, : T * 81]
                o2v = o2.rearrange("p (t a j) -> p t a j", a=9, j=9)
                E.tensor_tensor(
                    out=ov, in0=hv[:, :, 0:9, :], in1=be[0],
                    op=mybir.AluOpType.mult,
                )
                E.tensor_tensor(
                    out=o2v, in0=hv[:, :, 1:10, :], in1=be[1],
                    op=mybir.AluOpType.mult,
                )
                E.tensor_add(out=ov, in0=ov, in1=o2v)
                E.tensor_tensor(
                    out=o2v, in0=hv[:, :, 2:11, :], in1=be[2],
                    op=mybir.AluOpType.mult,
                )
                E.tensor_add(out=ov, in0=ov, in1=o2v)
                # --- L1 = 2x2 pool of the band, then standard 2-tap blend ---
                pl = blpool.tile([P, 5 * 200], F16, name="pl")[:, : T * 200]
                plv = pl.rearrange("p (t u s) -> p t u s", u=10, s=20)
                E.tensor_tensor(
                    out=plv, in0=B[:, :, 0:20:2, 0:20], in1=B[:, :, 1:20:2, 0:20],
                    op=mybir.AluOpType.add,
                )
                b1t = blpool.tile([P, 5 * 100], F16, name="b1t")[:, : T * 100]
                b1v = b1t.rearrange("p (t u v) -> p t u v", u=10, v=10)
                E.tensor_tensor(
                    out=b1v, in0=plv[:, :, :, 0:20:2], in1=plv[:, :, :, 1:20:2],
                    op=mybir.AluOpType.add,
                )
                blend2tap(
                    E, b1v[:, :, 0:10, 0:9], b1v[:, :, 0:10, 1:10], T,
                    0 * NT + t0, (1 * NT + t0) * 81,
                )
                # --- L2 / L3 from their gathered bands ---
                for gi, l in ((1, 2), (2, 3)):
                    s_in = SY[l]
                    bwv = bg[:, :, BOFF[l] : BOFF[l] + 10 * s_in].rearrange(
                        "p t (r s) -> p t r s", s=s_in
                    )
                    blend2tap(
                        E, bwv[:, :, 0:10, 0:9], bwv[:, :, 0:10, 1:10], T,
                        gi * NT + t0, (l * NT + t0) * 81,
                    )

            def stage_out(g):
                T = TG[g]
                t0 = GOF[g]
                ov = outp[:].rearrange("p (l x) -> p l x", l=NLVL)[
                    :, :, t0 * 81 : (t0 + T) * 81
                ]
                sv = out_sb[:].rearrange("p (l x) -> p l x", l=NLVL)[
                    :, :, t0 * 81 : (t0 + T) * 81
                ]
                nc.scalar.dma_start(ov, sv)

            with rep_ctx:
                # Queue discipline: Tensor=matmuls, ACT=loads+copies (+outs at
                # end), Sync=writes, Pool=gathers, DVE=blends only. Each
                # queue's instructions are in dependency order with no
                # back-edges, so no head-of-line blocking.
                bands = {}
                g_of_t = {}
                for g in range(len(TG)):
                    for tt in range(TG[g]):
                        g_of_t[GOF[g] + tt] = (g, tt)
                for t in range(NT):
                    stage_mm(t)
                    g, tt = g_of_t[t]
                    if tt == 0:
                        bands[g] = bpool.tile([P, 5 * BTOT], F16, name="band")[
                            :, : TG[g] * BTOT
                        ]
                    stage_gather_tile(t, bands[g], tt)
                    if tt == TG[g] - 1:
                        stage_blend(g, bands.pop(g))
                for g in range(len(TG)):
                    stage_out(g)

    nc.compile()
    return nc


# ---------------- host side ----------------

def _pool2(x):
    n, c, h, w = x.shape
    return x.reshape(n, c, h // 2, 2, w // 2, 2).mean(axis=(3, 5))


def _core_geom(c):
    """core -> (batch, y-base, x-base) of its 24x80 quadrant."""
    b = c // 4
    quad = c % 4
    return b, (quad // 2) * 24, (quad % 2) * 80


def _query_hw():
    """(t, p) -> (h, w) within a quadrant, vectorized [NT, P]."""
    t = np.arange(NT)[:, None]
    p = np.arange(P)[None, :]
    bh, bw = t // 5, t % 5
    r, cc = p // 16, p % 16
    return bh * 8 + r, bw * 16 + cc


def _host_prep(fmap1, fmap2, coords):
    import ml_dtypes

    fmap1 = np.asarray(fmap1, np.float32)
    fmap2 = np.asarray(fmap2, np.float32)
    coords = np.asarray(coords, np.float32)
    scale = np.float32(1.0 / np.sqrt(D))

    # pooled + scaled fmap2 levels
    levels = []
    cur = fmap2 * scale
    for l in range(NLVL):
        levels.append(cur)
        if l < NLVL - 1:
            cur = _pool2(cur)

    hq, wq = _query_hw()  # [NT, P]

    in_maps = []
    for c in range(NCORES):
        b, ybase, xbase = _core_geom(c)

        # --- windowed f2: L0 full-height x-window + merged L2|L3 blocks ---
        f2w = np.zeros((D, NPOSW), np.float32)
        wx0 = xbase - WPAD[0]
        xs = np.arange(wx0, wx0 + WXC[0])
        valid = (xs >= 0) & (xs < LW[0])
        blk = np.zeros((D, WXC[0], 48), np.float32)
        blk[:, valid, :] = levels[0][b][:, :, xs[valid]].transpose(0, 2, 1)
        f2w[:, 0 : WXC[0] * 48] = blk.reshape(D, -1)
        # merged L2-window | L3-map blocks, one per tile column bw
        l3flat = levels[3][b].reshape(D, -1)  # [D, 120]
        wx0 = (xbase >> 2) - WPAD[2]
        for bw in range(5):
            xs = np.arange(wx0 + 4 * bw + 2, wx0 + 4 * bw + 2 + 20)
            valid = (xs >= 0) & (xs < LW[2])
            blk = np.zeros((D, 20, SY[2]), np.float32)
            blk[:, valid, :] = levels[2][b][:, :, xs[valid]].transpose(0, 2, 1)
            o = LOFF23 + 360 * bw
            f2w[:, o : o + 240] = blk.reshape(D, -1)
            f2w[:, o + 240 : o + 360] = l3flat
        # device layout [P, 2, NPOSW]: partition p holds contraction rows
        # (p, P + p)
        f2c = np.ascontiguousarray(
            f2w.astype(ml_dtypes.bfloat16).reshape(2, P, NPOSW).transpose(1, 0, 2)
        ).reshape(P, 2 * NPOSW)

        # --- f1 in (t, k)-blocked layout: [P_contr, NT, 2, P_query] ---
        habs = ybase + hq  # [NT, P]
        wabs = xbase + wq
        f1c = fmap1[b][:, habs.ravel(), wabs.ravel()].reshape(2, P, NT, P)
        f1c = np.ascontiguousarray(
            f1c.astype(ml_dtypes.bfloat16).transpose(1, 2, 0, 3)
        ).reshape(P, NT * 2 * P)

        # --- per-query lookup indices and separable blend weights ---
        cx = coords[b, 0, habs, wabs]  # [NT, P]
        cy = coords[b, 1, habs, wabs]
        tgrid = np.arange(NT)[:, None]
        bwt = tgrid % 5
        p_arr = np.arange(P)[None, :]

        idx_all = np.zeros((3, NT, P), np.int64)  # gathered levels L0m/L2/L3
        ab0 = np.zeros((NT, P, 54), np.float32)   # merged L0 alpha/beta
        ab_all = np.zeros((3, NT, P, 36), np.float32)  # L1, L2, L3
        rr = np.arange(10)

        def lvl(l):
            inv = np.float32(1.0 / (1 << l))
            x = cx * inv
            y = cy * inv
            x0 = np.floor(x)
            y0 = np.floor(y)
            wx = (x - x0).astype(np.float32)
            wy = (y - y0).astype(np.float32)
            x0i = x0.astype(np.int64)
            y0i = y0.astype(np.int64)
            vx = ((x0i[..., None] + rr - 4) >= 0) & (
                (x0i[..., None] + rr - 4) <= LW[l] - 1
            )
            vy = ((y0i[..., None] + rr - 4) >= 0) & (
                (y0i[..., None] + rr - 4) <= LH[l] - 1
            )
            return x0i, y0i, wx, wy, vx, vy

        x00, y00, wx0_, wy0, vx0, vy0 = lvl(0)
        x01, y01, wx1, wy1, vx1, vy1 = lvl(1)
        base0 = p_arr * SCOLS

        # merged L0/L1 band: anchor (2*x01-8, 2*y01-8), parity-absorbed taps
        ox0 = xbase + 16 * bwt - ML[0]
        relxa = np.clip(2 * x01 - 8 - ox0, -10, SXB0 - 20)
        relya = np.clip(2 * y01 - 8, -24, 54)
        idx_all[0] = base0 + SOFFR[0] + relxa * 48 + relya
        px = (x00 - 2 * x01).astype(np.int64)  # {0, 1}
        py = (y00 - 2 * y01).astype(np.int64)
        ay0 = vy0[..., 0:9] * (1.0 - wy0)[..., None]
        ay1 = vy0[..., 1:10] * wy0[..., None]
        bx0 = vx0[..., 0:9] * (1.0 - wx0_)[..., None]
        bx1 = vx0[..., 1:10] * wx0_[..., None]
        m0 = (py == 0)[..., None]
        ab0[:, :, 0:9] = np.where(m0, ay0, 0.0)
        ab0[:, :, 9:18] = np.where(m0, ay1, ay0)
        ab0[:, :, 18:27] = np.where(m0, 0.0, ay1)
        m1 = (px == 0)[..., None]
        ab0[:, :, 27:36] = np.where(m1, bx0, 0.0)
        ab0[:, :, 36:45] = np.where(m1, bx1, bx0)
        ab0[:, :, 45:54] = np.where(m1, 0.0, bx1)

        # L1 (pooled on device; 0.25 pool scale folded into a-taps)
        ab_all[0, :, :, 0:9] = 0.25 * vy1[..., 0:9] * (1.0 - wy1)[..., None]
        ab_all[0, :, :, 9:18] = 0.25 * vy1[..., 1:10] * wy1[..., None]
        ab_all[0, :, :, 18:27] = vx1[..., 0:9] * (1.0 - wx1)[..., None]
        ab_all[0, :, :, 27:36] = vx1[..., 1:10] * wx1[..., None]

        # L2 x-major bbox / L3 full-map y-major, as before
        for gi, l in ((1, 2), (2, 3)):
            x0i, y0i, wx, wy, vx, vy = lvl(l)
            base = base0 + SOFFR[l]
            if l == 2:
                oxabs = (xbase >> l) + ((16 >> l) * bwt) - ML[l]
                relx = np.clip(x0i - 4 - oxabs, -10, SXB[l] + 6)
                rely = np.clip(y0i - 4, -9, SY[l])
                idx_all[gi] = base + relx * SY[l] + rely
                ab_all[gi, :, :, 0:9] = vy[..., 0:9] * (1.0 - wy)[..., None]
                ab_all[gi, :, :, 9:18] = vy[..., 1:10] * wy[..., None]
                ab_all[gi, :, :, 18:27] = vx[..., 0:9] * (1.0 - wx)[..., None]
                ab_all[gi, :, :, 27:36] = vx[..., 1:10] * wx[..., None]
            else:
                x0c = np.clip(x0i, -5, LW[l] + 4)
                y0c = np.clip(y0i, -5, LH[l] + 4)
                idx_all[gi] = base + (y0c - 4) * LW[l] + (x0c - 4)
                ab_all[gi, :, :, 0:9] = vx[..., 0:9] * (1.0 - wx)[..., None]
                ab_all[gi, :, :, 9:18] = vx[..., 1:10] * wx[..., None]
                ab_all[gi, :, :, 18:27] = vy[..., 0:9] * (1.0 - wy)[..., None]
                ab_all[gi, :, :, 27:36] = vy[..., 1:10] * wy[..., None]

        abt = np.concatenate(
            [
                ab0.transpose(1, 0, 2).reshape(P, -1),
                ab_all.transpose(2, 0, 1, 3).reshape(P, -1),
            ],
            axis=1,
        ).astype(np.float16)
        in_maps.append({
            "f1t": f1c,
            "f2t": np.ascontiguousarray(f2c),
            # [P, gathered-level-major, t-contig]
            "idxt": np.ascontiguousarray(
                idx_all.astype(np.int32).transpose(2, 0, 1).reshape(P, -1)
            ),
            "abt": np.ascontiguousarray(abt),
        })
    return in_maps


def assemble(results):
    out = np.empty((B, NLVL * 81, H, W), np.float32)
    hq, wq = _query_hw()
    for c in range(NCORES):
        b, ybase, xbase = _core_geom(c)
        r = np.asarray(results[c]["outp"], np.float32).reshape(P, NLVL, NT, 81)
        blk = r.transpose(1, 3, 2, 0)  # [NLVL, 81, NT, P]
        # L3 channel blocks are (y-tap, x-tap); reference wants (x-tap, y-tap)
        l3 = blk[3].reshape(9, 9, NT, P).transpose(1, 0, 2, 3).reshape(81, NT, P)
        blk = np.concatenate([blk[0:3], l3[None]], axis=0)
        out[b, :, ybase + hq, xbase + wq] = blk.reshape(NLVL * 81, NT, P).transpose(
            1, 2, 0
        )
    return out


_NC_CACHE = {}


def get_nc():
    if "nc" not in _NC_CACHE:
        _NC_CACHE["nc"] = build_nc()
    return _NC_CACHE["nc"]


def kernel(fmap1, fmap2, coords):
    in_maps = _host_prep(fmap1, fmap2, coords)
    nc = get_nc()
    res = run_bass_kernel_spmd(nc, in_maps, core_ids=list(range(NCORES)))
    return assemble(res.results)


# revision 61
# speedup vs baseline: 1.1404x; 1.1404x over previous
"""Trainium2 Bass kernel for nn_CSFlow (RAFT-style correlation pyramid lookup).

v10: separable blend weights + grouped blends + lean DMA schedule.

Structure (per core, one 24h x 80w quadrant of one batch):
  - corr(q, pos) = <fmap1[:, q], fmap2[:, pos]> / sqrt(D); pooling folded into
    pooled fmap2 levels (linearity), one matmul per pyramid level slice.
  - 15 tiles of 8h x 16w query blocks (128 queries = partitions). Per tile a
    static per-level x-range bbox (2792 cols total) is matmul'd, copied
    PSUM->SBUF f16, written to a DRAM scratch, and 4 indirect gathers pull
    each query's band (one per level; HW honors one offset per partition).
  - Blends use separable weights: per (level, tile, query) vectors
    a0/a1[9] (inner-tap weight x validity) and b0/b1[9] (outer-tap), applied
    as broadcast tensor_tensor ops batched over groups of tiles:
      h[t,r,j]  = g0*a0 + g1*a1   (3 ops per level-group)
      out[t,a,j] = h[.,a,.]*b0 + h[.,a+1,.]*b1  (3 ops, into persistent out)
  - Outputs accumulate in one SBUF tile; one DMA per tile group.
  - Guard zones of the scratch are zeroed in one DMA upfront.
  - Input loads are ordered so tile 0's write (the head of the serial
    gather chain on the Pool engine) lands as early as possible.

Output channel order per level block l: a*9+j where a = x-tap, j = y-tap for
L0-2; L3 blocks are (y-tap major) and host transposes them.
"""

import numpy as np

import concourse.bass as bass
import concourse.mybir as mybir
import concourse.tile as tile
from concourse import bacc
from concourse.bass_utils import run_bass_kernel_spmd

# problem shape (hardcoded per harness contract)
B, D, H, W = 2, 256, 48, 160
NCORES = 8
P = 128
NT = 15                      # tiles per core (3x5 blocks of 8h x 16w)
NLVL = 4
QPC = NT * P                 # 1920 queries per core (24h x 80w quadrant)
LH = [48, 24, 12, 6]
LW = [160, 80, 40, 20]

# L0+L1 are served by ONE merged gather per (tile, query): a 20x20
# L0-granularity band anchored at (2*floor(cx/2)-8, 2*floor(cy/2)-8) covers
# both L0's 10x10 window and the 2x2-pool preimage of L1's 10x10 window.
# L1 values are produced on-device by pooling the band; the anchor parity
# (px, py) is absorbed into 3-tap blend stages for L0.
SY = [48, 24, 12, 20]        # L0m inner stride: full 48 rows (L1 preimage)
SXB0 = 54                    # L0m bbox x-extent
SXB = [54, 28, 20, 6]        # bbox outer extent (L2/L3 unchanged)
COLS = [2592, 0, 240, 120]   # bbox positions per level (L1 merged into L0)
# per-partition section order [L2 240 | L3 120 | padh 144 | L0m 2592 |
# padt 40]: zero pads absorb the merged band's worst-case under/overrun so
# every read lands on written scratch -> no guard zones. Pads are zeroed in
# the staging tiles once per ring buffer and rewritten with each tile.
PADH = 144
PADT = 40
SCOLS = 240 + 120 + PADH + 2592 + PADT  # 3136 per-partition cols
SOFFR = [360 + PADH, -1, 0, 240]  # per-partition section offsets by level

WXC = [118, 0, 40, 120]      # windowed-f2 x-columns per level (L1 merged away)
# f2 layout: [L0 region 118x48 | 5 merged L2|L3 blocks of 360]
LOFF23 = 118 * 48            # 5664
NPOSW = LOFF23 + 5 * 360     # 7464
WPAD = [14, 12, 10, 0]       # f2-window left margin vs quadrant x-base
ML = [14, 10, 8, 0]          # bbox left margin vs block x-origin (per level)

SCR1 = P * SCOLS  # no guard zones (see SOFFR comment)
BLEN = [19 * 48 + 20, 0, 118, 190]  # gather band lengths (L0m: 20x20 block)
BOFF = [0, -1, 932, 1052]    # band section offsets within a tile's band
BTOT = 1252  # incl. view-overhang padding (L2 +2 and L3's [10,20] window)
GLVL = [0, 2, 3]             # gathered levels

TG = [5, 5, 3, 2]            # blend/output tile groups (last small = short tail)
GOF = [0, 5, 10, 13]

F16 = mybir.dt.float16
F32 = mybir.dt.float32
BF16 = mybir.dt.bfloat16
I32 = mybir.dt.int32

MM_CHUNK = 512
# psum chunks: (name, [(level, col-off-in-level, psum-off, size)...], total,
#               copy engine: 0=DVE 1=ACT)
# psum chunks; stag cols via SOFFR: c0-c2 [504:3096] (L0m), c3 [0:360]
# (L2|L3). All copies on ACT; write A ([504:3136], after c2, incl. padt)
# unblocks the L0m gather; write B ([0:504], after c3, incl. padh) unblocks
# L2/L3.
CHUNKS = [
    ("c0", [(0, 0, 0, 1024)], 1024),
    ("c1", [(0, 1024, 0, 1024)], 1024),
    ("c2", [(0, 2048, 0, 544)], 544),
    ("c3", [(2, 0, 0, 360)], 360),
]


def _f2_slice_start(l, bw):
    """Static f2-window column offset for tile column bw (0..4), level l."""
    if l == 0:
        return (16 * bw) * SY[0]
    # merged L2-window + L3-map block, replicated per tile column
    return LOFF23 + 360 * bw


def build_nc(repeat=1):
    nc = bacc.Bacc("TRN2", target_bir_lowering=False, debug=False)

    # f1 is (t, k)-blocked so every load and matmul lhsT slice is contiguous
    f1t = nc.dram_tensor("f1t", [P, NT * 2 * P], BF16, kind="ExternalInput")
    f2t = nc.dram_tensor("f2t", [P, 2 * NPOSW], BF16, kind="ExternalInput")
    # l-major t-contig (l in GLVL order: L0m, L2, L3)
    idxt = nc.dram_tensor("idxt", [P, 3 * NT], I32, kind="ExternalInput")
    # separable blend weights, l-major t-contig:
    #   L0m: alpha[3][9] beta[3][9] (54); L1/L2/L3: a0 a1 b0 b1 (36 each)
    abt = nc.dram_tensor("abt", [P, NT * 162], F16, kind="ExternalInput")
    # l-major: [P, NLVL, NT, 81]
    outp = nc.dram_tensor("outp", [P, NLVL * NT * 81], F16, kind="ExternalOutput")

    with tile.TileContext(nc) as tc:
        with (
            tc.tile_pool(name="dram", bufs=1, space="DRAM") as dpool,
            tc.tile_pool(name="const", bufs=1) as cpool,
            tc.tile_pool(name="stag", bufs=3) as stpool,
            tc.tile_pool(name="bands", bufs=2) as bpool,
            tc.tile_pool(name="blend", bufs=4) as blpool,
            tc.tile_pool(name="psum", bufs=4, space="PSUM") as pspool,
        ):
            # one DRAM scratch tensor PER TILE: an indirect gather's source AP
            # must be a whole tensor (offset 0), and a shared tensor would give
            # every later write a false WAR dependency on all prior gathers.
            scrt = [dpool.tile([SCR1], F16, name=f"scrt{t}") for t in range(NT)]

            # f1sb mirrors f1t's (t, k)-blocked layout
            f1sb = cpool.tile([P, NT * 2 * P], BF16)
            f2sb = cpool.tile([P, 2 * NPOSW], BF16)
            idx_sb = cpool.tile([P, 3 * NT], I32)
            ab_sb = cpool.tile([P, NT * 162], F16)
            # l-major: [P, NLVL, NT, 81]
            out_sb = cpool.tile([P, NLVL * NT * 81], F16)

            # --- critical-path loads first (finest useful granularity so
            # tile 0's chunks unblock ASAP); every load is a contiguous
            # [128, N] row copy. ---
            def f2load(eng, a, b):
                for k in range(2):
                    eng.dma_start(
                        f2sb[:, k * NPOSW + a : k * NPOSW + b],
                        f2t[:, k * NPOSW + a : k * NPOSW + b],
                    )

            # tile 0's critical path, ALONE on the sync queue in need-order so
            # no bulk load competes for HBM bandwidth before tile 0 streams
            f2load(nc.sync, 0, 1024)
            nc.sync.dma_start(f1sb[:, 0 : 4 * P], f1t[:, 0 : 4 * P])  # tiles 0-1
            nc.sync.dma_start(idx_sb[:], idxt[:])
            f2load(nc.sync, 1024, 2592)
            f2load(nc.sync, LOFF23, LOFF23 + 360)

            # rest of f1 + f2 + weights, staged behind the pipeline
            def deferred_loads(t):
                if t == 0:
                    with tc.high_priority(offset=-100):
                        f2load(nc.scalar, 2592, 3360)
                        f2load(nc.scalar, LOFF23 + 360, NPOSW)
                elif t == 1:
                    with tc.high_priority(offset=-100):
                        nc.sync.dma_start(
                            f1sb[:, 4 * P : NT * 2 * P], f1t[:, 4 * P : NT * 2 * P]
                        )
                        f2load(nc.scalar, 3360, LOFF23)
                elif t == 2:
                    with tc.high_priority(offset=-100):
                        nc.scalar.dma_start(ab_sb[:], abt[:])

            import contextlib

            rep_ctx = tc.For_i(0, repeat, 1) if repeat > 1 else contextlib.nullcontext()

            def stage_mm(t):
                """matmuls -> psum -> f16 staging (ACT) -> two scratch writes."""
                # deferred loads must be EMITTED before this tile's readers
                # or the read-before-write flips the hazard direction
                deferred_loads(t)
                bw = t % 5
                stag = stpool.tile([P, SCOLS], F16, name="stag")
                if t < 3:
                    # zero the pad columns once per ring buffer; copies never
                    # touch them, so they persist for later tiles
                    nc.vector.memset(stag[:, 360 : 360 + PADH], 0.0)
                    nc.vector.memset(stag[:, SCOLS - PADT : SCOLS], 0.0)
                wv = scrt[t][:].rearrange("(p x) -> p x", x=SCOLS)
                for ci, (nm, parts, csz) in enumerate(CHUNKS):
                    ps = pspool.tile([P, 1024], F32, name="cps")[:, :csz]
                    for k in range(2):
                        for (l, coff, poff, sz) in parts:
                            fs = k * NPOSW + _f2_slice_start(l, bw) + coff
                            for soff in range(0, sz, MM_CHUNK):
                                ssz = min(MM_CHUNK, sz - soff)
                                nc.tensor.matmul(
                                    ps[:, poff + soff : poff + soff + ssz],
                                    f1sb[:, (2 * t + k) * P : (2 * t + k + 1) * P],
                                    f2sb[:, fs + soff : fs + soff + ssz],
                                    start=(k == 0),
                                    stop=(k == 1),
                                )
                    soff0 = SOFFR[parts[0][0]] + parts[0][1]
                    nc.scalar.copy(stag[:, soff0 : soff0 + csz], ps)
                    if ci == 2:
                        # L0m staged -> write A (incl. padt), unblocking its
                        # gather
                        nc.sync.dma_start(wv[:, 504:SCOLS], stag[:, 504:SCOLS])
                    elif ci == 3:
                        # L2|L3 (+padh) -> write B
                        nc.sync.dma_start(wv[:, 0:504], stag[:, 0:504])

            def stage_gather_tile(t, band, tt):
                """indirect gathers (L0m, L2, L3) for tile t into band slot tt."""
                for gi, l in enumerate(GLVL):
                    nc.gpsimd.indirect_dma_start(
                        out=band[
                            :, tt * BTOT + BOFF[l] : tt * BTOT + BOFF[l] + BLEN[l]
                        ],
                        out_offset=None,
                        in_=scrt[t][:].unsqueeze(1),
                        in_offset=bass.IndirectOffsetOnAxis(
                            ap=idx_sb[:, gi * NT + t : gi * NT + t + 1],
                            axis=0,
                        ),
                        element_offset=0,
                    )

            # ab sections: L0m [0:54*NT], L1 [54*NT:90*NT], L2 [...], L3
            ab0v = ab_sb[:, 0 : 54 * NT].rearrange("p (c v) -> p c v", v=54)
            abv = ab_sb[:, 54 * NT :].rearrange("p (c v) -> p c v", v=36)

            def blend2tap(E, g0, g1, T, c0i, os):
                """standard separable 2-tap blend into out_sb at offset os."""
                a0 = abv[:, c0i : c0i + T, 0:9].unsqueeze(2).to_broadcast(
                    (P, T, 10, 9)
                )
                a1 = abv[:, c0i : c0i + T, 9:18].unsqueeze(2).to_broadcast(
                    (P, T, 10, 9)
                )
                b0 = abv[:, c0i : c0i + T, 18:27].unsqueeze(3).to_broadcast(
                    (P, T, 9, 9)
                )
                b1 = abv[:, c0i : c0i + T, 27:36].unsqueeze(3).to_broadcast(
                    (P, T, 9, 9)
                )
                h = blpool.tile([P, 5 * 90], F16, name="h")[:, : T * 90]
                h2 = blpool.tile([P, 5 * 90], F16, name="h2")[:, : T * 90]
                hv = h.rearrange("p (t r j) -> p t r j", r=10, j=9)
                h2v = h2.rearrange("p (t r j) -> p t r j", r=10, j=9)
                E.tensor_tensor(out=hv, in0=g0, in1=a0, op=mybir.AluOpType.mult)
                E.tensor_tensor(out=h2v, in0=g1, in1=a1, op=mybir.AluOpType.mult)
                E.tensor_add(out=h, in0=h, in1=h2)
                ov = out_sb[:, os : os + T * 81].rearrange(
                    "p (t a j) -> p t a j", a=9, j=9
                )
                o2 = blpool.tile([P, 5 * 81], F16, name="o2")[:, : T * 81]
                o2v = o2.rearrange("p (t a j) -> p t a j", a=9, j=9)
                E.tensor_tensor(
                    out=ov, in0=hv[:, :, 0:9, :], in1=b0, op=mybir.AluOpType.mult
                )
                E.tensor_tensor(
                    out=o2v, in0=hv[:, :, 1:10, :], in1=b1, op=mybir.AluOpType.mult
                )
                E.tensor_add(out=ov, in0=ov, in1=o2v)

            def stage_blend(g, band):
                T = TG[g]
                t0 = GOF[g]
                E = nc.vector
                bg = band[:].rearrange("p (t x) -> p t x", t=T)
                # --- merged L0: 3-tap separable blend over the 20x20 band ---
                # view envelope 20*48=960 spills into the L2 band section; only
                # r<20, s<16 are read
                B = bg[:, :, 0:960].rearrange("p t (r s) -> p t r s", s=48)
                al = [
                    ab0v[:, t0 : t0 + T, 9 * k : 9 * k + 9]
                    .unsqueeze(2)
                    .to_broadcast((P, T, 11, 9))
                    for k in range(3)
                ]
                be = [
                    ab0v[:, t0 : t0 + T, 27 + 9 * k : 36 + 9 * k]
                    .unsqueeze(3)
                    .to_broadcast((P, T, 9, 9))
                    for k in range(3)
                ]
                h = blpool.tile([P, 5 * 99], F16, name="hm")[:, : T * 99]
                h2 = blpool.tile([P, 5 * 99], F16, name="hm2")[:, : T * 99]
                hv = h.rearrange("p (t r j) -> p t r j", r=11, j=9)
                h2v = h2.rearrange("p (t r j) -> p t r j", r=11, j=9)
                E.tensor_tensor(
                    out=hv, in0=B[:, :, 4:15, 4:13], in1=al[0],
                    op=mybir.AluOpType.mult,
                )
                E.tensor_tensor(
                    out=h2v, in0=B[:, :, 4:15, 5:14], in1=al[1],
                    op=mybir.AluOpType.mult,
                )
                E.tensor_add(out=h, in0=h, in1=h2)
                E.tensor_tensor(
                    out=h2v, in0=B[:, :, 4:15, 6:15], in1=al[2],
                    op=mybir.AluOpType.mult,
                )
                E.tensor_add(out=h, in0=h, in1=h2)
                os0 = (0 * NT + t0) * 81
                ov = out_sb[:, os0 : os0 + T * 81].rearrange(
                    "p (t a j) -> p t a j", a=9, j=9
                )
                o2 = blpool.tile([P, 5 * 81], F16, name="om2")[:, : T * 81]
                o2v = o2.rearrange("p (t a j) -> p t a j", a=9, j=9)
                E.tensor_tensor(
                    out=ov, in0=hv[:, :, 0:9, :], in1=be[0],
                    op=mybir.AluOpType.mult,
                )
                E.tensor_tensor(
                    out=o2v, in0=hv[:, :, 1:10, :], in1=be[1],
                    op=mybir.AluOpType.mult,
                )
                E.tensor_add(out=ov, in0=ov, in1=o2v)
                E.tensor_tensor(
                    out=o2v, in0=hv[:, :, 2:11, :], in1=be[2],
                    op=mybir.AluOpType.mult,
                )
                E.tensor_add(out=ov, in0=ov, in1=o2v)
                # --- L1 = 2x2 pool of the band, then standard 2-tap blend ---
                pl = blpool.tile([P, 5 * 200], F16, name="pl")[:, : T * 200]
                plv = pl.rearrange("p (t u s) -> p t u s", u=10, s=20)
                E.tensor_tensor(
                    out=plv, in0=B[:, :, 0:20:2, 0:20], in1=B[:, :, 1:20:2, 0:20],
                    op=mybir.AluOpType.add,
                )
                b1t = blpool.tile([P, 5 * 100], F16, name="b1t")[:, : T * 100]
                b1v = b1t.rearrange("p (t u v) -> p t u v", u=10, v=10)
                E.tensor_tensor(
                    out=b1v, in0=plv[:, :, :, 0:20:2], in1=plv[:, :, :, 1:20:2],
                    op=mybir.AluOpType.add,
                )
                blend2tap(
                    E, b1v[:, :, 0:10, 0:9], b1v[:, :, 0:10, 1:10], T,
                    0 * NT + t0, (1 * NT + t0) * 81,
                )
                # --- L2 / L3 from their gathered bands ---
                for gi, l in ((1, 2), (2, 3)):
                    s_in = SY[l]
                    bwv = bg[:, :, BOFF[l] : BOFF[l] + 10 * s_in].rearrange(
                        "p t (r s) -> p t r s", s=s_in
                    )
                    blend2tap(
                        E, bwv[:, :, 0:10, 0:9], bwv[:, :, 0:10, 1:10], T,
                        gi * NT + t0, (l * NT + t0) * 81,
                    )

            def stage_out(g):
                T = TG[g]
                t0 = GOF[g]
                ov = outp[:].rearrange("p (l x) -> p l x", l=NLVL)[
                    :, :, t0 * 81 : (t0 + T) * 81
                ]
                sv = out_sb[:].rearrange("p (l x) -> p l x", l=NLVL)[
                    :, :, t0 * 81 : (t0 + T) * 81
                ]
                nc.scalar.dma_start(ov, sv)

            with rep_ctx:
                # Queue discipline: Tensor=matmuls, ACT=loads+copies (+outs at
                # end), Sync=writes, Pool=gathers, DVE=blends only. Each
                # queue's instructions are in dependency order with no
                # back-edges, so no head-of-line blocking.
                bands = {}
                g_of_t = {}
                for g in range(len(TG)):
                    for tt in range(TG[g]):
                        g_of_t[GOF[g] + tt] = (g, tt)
                for t in range(NT):
                    stage_mm(t)
                    g, tt = g_of_t[t]
                    if tt == 0:
                        bands[g] = bpool.tile([P, 5 * BTOT], F16, name="band")[
                            :, : TG[g] * BTOT
                        ]
                    stage_gather_tile(t, bands[g], tt)
                    if tt == TG[g] - 1:
                        stage_blend(g, bands.pop(g))
                for g in range(len(TG)):
                    stage_out(g)

    nc.compile()
    return nc


# ---------------- host side ----------------

def _pool2(x):
    n, c, h, w = x.shape
    return x.reshape(n, c, h // 2, 2, w // 2, 2).mean(axis=(3, 5))


def _core_geom(c):
    """core -> (batch, y-base, x-base) of its 24x80 quadrant."""
    b = c // 4
    quad = c % 4
    return b, (quad // 2) * 24, (quad % 2) * 80


def _query_hw():
    """(t, p) -> (h, w) within a quadrant, vectorized [NT, P]."""
    t = np.arange(NT)[:, None]
    p = np.arange(P)[None, :]
    bh, bw = t // 5, t % 5
    r, cc = p // 16, p % 16
    return bh * 8 + r, bw * 16 + cc


def _host_prep(fmap1, fmap2, coords):
    import ml_dtypes

    fmap1 = np.asarray(fmap1, np.float32)
    fmap2 = np.asarray(fmap2, np.float32)
    coords = np.asarray(coords, np.float32)
    scale = np.float32(1.0 / np.sqrt(D))

    # pooled + scaled fmap2 levels
    levels = []
    cur = fmap2 * scale
    for l in range(NLVL):
        levels.append(cur)
        if l < NLVL - 1:
            cur = _pool2(cur)

    hq, wq = _query_hw()  # [NT, P]

    in_maps = []
    for c in range(NCORES):
        b, ybase, xbase = _core_geom(c)

        # --- windowed f2: L0 full-height x-window + merged L2|L3 blocks ---
        f2w = np.zeros((D, NPOSW), np.float32)
        wx0 = xbase - WPAD[0]
        xs = np.arange(wx0, wx0 + WXC[0])
        valid = (xs >= 0) & (xs < LW[0])
        blk = np.zeros((D, WXC[0], 48), np.float32)
        blk[:, valid, :] = levels[0][b][:, :, xs[valid]].transpose(0, 2, 1)
        f2w[:, 0 : WXC[0] * 48] = blk.reshape(D, -1)
        # merged L2-window | L3-map blocks, one per tile column bw
        l3flat = levels[3][b].reshape(D, -1)  # [D, 120]
        wx0 = (xbase >> 2) - WPAD[2]
        for bw in range(5):
            xs = np.arange(wx0 + 4 * bw + 2, wx0 + 4 * bw + 2 + 20)
            valid = (xs >= 0) & (xs < LW[2])
            blk = np.zeros((D, 20, SY[2]), np.float32)
            blk[:, valid, :] = levels[2][b][:, :, xs[valid]].transpose(0, 2, 1)
            o = LOFF23 + 360 * bw
            f2w[:, o : o + 240] = blk.reshape(D, -1)
            f2w[:, o + 240 : o + 360] = l3flat
        # device layout [P, 2, NPOSW]: partition p holds contraction rows
        # (p, P + p)
        f2c = np.ascontiguousarray(
            f2w.astype(ml_dtypes.bfloat16).reshape(2, P, NPOSW).transpose(1, 0, 2)
        ).reshape(P, 2 * NPOSW)

        # --- f1 in (t, k)-blocked layout: [P_contr, NT, 2, P_query] ---
        habs = ybase + hq  # [NT, P]
        wabs = xbase + wq
        f1c = fmap1[b][:, habs.ravel(), wabs.ravel()].reshape(2, P, NT, P)
        f1c = np.ascontiguousarray(
            f1c.astype(ml_dtypes.bfloat16).transpose(1, 2, 0, 3)
        ).reshape(P, NT * 2 * P)

        # --- per-query lookup indices and separable blend weights ---
        cx = coords[b, 0, habs, wabs]  # [NT, P]
        cy = coords[b, 1, habs, wabs]
        tgrid = np.arange(NT)[:, None]
        bwt = tgrid % 5
        p_arr = np.arange(P)[None, :]

        idx_all = np.zeros((3, NT, P), np.int64)  # gathered levels L0m/L2/L3
        ab0 = np.zeros((NT, P, 54), np.float32)   # merged L0 alpha/beta
        ab_all = np.zeros((3, NT, P, 36), np.float32)  # L1, L2, L3
        rr = np.arange(10)

        def lvl(l):
            inv = np.float32(1.0 / (1 << l))
            x = cx * inv
            y = cy * inv
            x0 = np.floor(x)
            y0 = np.floor(y)
            wx = (x - x0).astype(np.float32)
            wy = (y - y0).astype(np.float32)
            x0i = x0.astype(np.int64)
            y0i = y0.astype(np.int64)
            vx = ((x0i[..., None] + rr - 4) >= 0) & (
                (x0i[..., None] + rr - 4) <= LW[l] - 1
            )
            vy = ((y0i[..., None] + rr - 4) >= 0) & (
                (y0i[..., None] + rr - 4) <= LH[l] - 1
            )
            return x0i, y0i, wx, wy, vx, vy

        x00, y00, wx0_, wy0, vx0, vy0 = lvl(0)
        x01, y01, wx1, wy1, vx1, vy1 = lvl(1)
        base0 = p_arr * SCOLS

        # merged L0/L1 band: anchor (2*x01-8, 2*y01-8), parity-absorbed taps
        ox0 = xbase + 16 * bwt - ML[0]
        relxa = np.clip(2 * x01 - 8 - ox0, -10, SXB0 - 20)
        relya = np.clip(2 * y01 - 8, -24, 54)
        idx_all[0] = base0 + SOFFR[0] + relxa * 48 + relya
        px = (x00 - 2 * x01).astype(np.int64)  # {0, 1}
        py = (y00 - 2 * y01).astype(np.int64)
        ay0 = vy0[..., 0:9] * (1.0 - wy0)[..., None]
        ay1 = vy0[..., 1:10] * wy0[..., None]
        bx0 = vx0[..., 0:9] * (1.0 - wx0_)[..., None]
        bx1 = vx0[..., 1:10] * wx0_[..., None]
        m0 = (py == 0)[..., None]
        ab0[:, :, 0:9] = np.where(m0, ay0, 0.0)
        ab0[:, :, 9:18] = np.where(m0, ay1, ay0)
        ab0[:, :, 18:27] = np.where(m0, 0.0, ay1)
        m1 = (px == 0)[..., None]
        ab0[:, :, 27:36] = np.where(m1, bx0, 0.0)
        ab0[:, :, 36:45] = np.where(m1, bx1, bx0)
        ab0[:, :, 45:54] = np.where(m1, 0.0, bx1)

        # L1 (pooled on device; 0.25 pool scale folded into a-taps)
        ab_all[0, :, :, 0:9] = 0.25 * vy1[..., 0:9] * (1.0 - wy1)[..., None]
        ab_all[0, :, :, 9:18] = 0.25 * vy1[..., 1:10] * wy1[..., None]
        ab_all[0, :, :, 18:27] = vx1[..., 0:9] * (1.0 - wx1)[..., None]
        ab_all[0, :, :, 27:36] = vx1[..., 1:10] * wx1[..., None]

        # L2 x-major bbox / L3 full-map y-major, as before
        for gi, l in ((1, 2), (2, 3)):
            x0i, y0i, wx, wy, vx, vy = lvl(l)
            base = base0 + SOFFR[l]
            if l == 2:
                oxabs = (xbase >> l) + ((16 >> l) * bwt) - ML[l]
                relx = np.clip(x0i - 4 - oxabs, -10, SXB[l] + 6)
                rely = np.clip(y0i - 4, -9, SY[l])
                idx_all[gi] = base + relx * SY[l] + rely
                ab_all[gi, :, :, 0:9] = vy[..., 0:9] * (1.0 - wy)[..., None]
                ab_all[gi, :, :, 9:18] = vy[..., 1:10] * wy[..., None]
                ab_all[gi, :, :, 18:27] = vx[..., 0:9] * (1.0 - wx)[..., None]
                ab_all[gi, :, :, 27:36] = vx[..., 1:10] * wx[..., None]
            else:
                x0c = np.clip(x0i, -5, LW[l] + 4)
                y0c = np.clip(y0i, -5, LH[l] + 4)
                idx_all[gi] = base + (y0c - 4) * LW[l] + (x0c - 4)
                ab_all[gi, :, :, 0:9] = vx[..., 0:9] * (1.0 - wx)[..., None]
                ab_all[gi, :, :, 9:18] = vx[..., 1:10] * wx[..., None]
                ab_all[gi, :, :, 18:27] = vy[..., 0:9] * (1.0 - wy)[..., None]
                ab_all[gi, :, :, 27:36] = vy[..., 1:10] * wy[..., None]

        abt = np.concatenate(
            [
                ab0.transpose(1, 0, 2).reshape(P, -1),
                ab_all.transpose(2, 0, 1, 3).reshape(P, -1),
            ],
            axis=1,
        ).astype(np.float16)
        in_maps.append({
            "f1t": f1c,
            "f2t": np.ascontiguousarray(f2c),
            # [P, gathered-level-major, t-contig]
            "idxt": np.ascontiguousarray(
                idx_all.astype(np.int32).transpose(2, 0, 1).reshape(P, -1)
            ),
            "abt": np.ascontiguousarray(abt),
        })
    return in_maps


def assemble(results):
    out = np.empty((B, NLVL * 81, H, W), np.float32)
    hq, wq = _query_hw()
    for c in range(NCORES):
        b, ybase, xbase = _core_geom(c)
        r = np.asarray(results[c]["outp"], np.float32).reshape(P, NLVL, NT, 81)
        blk = r.transpose(1, 3, 2, 0)  # [NLVL, 81, NT, P]
        # L3 channel blocks are (y-tap, x-tap); reference wants (x-tap, y-tap)
        l3 = blk[3].reshape(9, 9, NT, P).transpose(1, 0, 2, 3).reshape(81, NT, P)
        blk = np.concatenate([blk[0:3], l3[None]], axis=0)
        out[b, :, ybase + hq, xbase + wq] = blk.reshape(NLVL * 81, NT, P).transpose(
            1, 2, 0
        )
    return out


_NC_CACHE = {}


def get_nc():
    if "nc" not in _NC_CACHE:
        _NC_CACHE["nc"] = build_nc()
    return _NC_CACHE["nc"]


def kernel(fmap1, fmap2, coords):
    in_maps = _host_prep(fmap1, fmap2, coords)
    nc = get_nc()
    res = run_bass_kernel_spmd(nc, in_maps, core_ids=list(range(NCORES)))
    return assemble(res.results)
